# revision 32
# baseline (speedup 1.0000x reference)
"""CifarNetMem Trainium2 kernel: LUT-quantized CNN forward pass.

Distribution: pure batch data parallelism, 512 images -> 8 NeuronCores x 64.

Numerics: every post-quantization value is a lattice point k/16 (|k| <= 64).
Activations are stored scaled by 16 (integers 0..64) in bf16 (exact);
weights stay as lattice values (bf16-exact). bf16 matmuls with fp32 PSUM
accumulation are BIT-EXACT vs the fp32 reference for conv2..6 + fc1..3.
conv1's input x is split into three bf16 terms (h1+h2+h3 == x exactly), so
conv1 matches fp32 up to accumulation order (~1 ulp; a handful of images
land on a quantization tie and flip one lattice step - inherent fp32-order
ambiguity).

Convs as full-width [K<=128 x M=128] matmuls with HOST-BUILT BLOCK-DIAGONAL
weights: the 128 partitions carry several images' channel strips at once
(4x32 for conv1/2, 2x64 for conv3/4), so one instruction per (image-bank,
tap) replaces per-subarray tiles. Moving operands are tight [y, x] strided
views of zero-framed per-image layouts (34x34 / 18x18 / 10x10), so each
matmul streams exactly the output pixels. conv1 packs dy + the 3 bf16
splits into K (27 rows/strip; dx = 3 accumulating passes). conv3/conv5 use
TAP-PAIRING: an SBUF->SBUF DMA builds a one-frame-row-shifted replica of
the input on spare partitions (in Xt/A1's dead space), so K covers taps
(dy, dy+1) together: 6 passes instead of 9. fc1's bias is added in-PSUM
via a K=1 ones-row matmul so all 4 output chunks share one evac chain.
Per-round 4-bank PSUM tiles let one ACT/DVE/GPSIMD chain drain a round;
input X is DMA'd in 4 chunks overlapping conv1; fw1 is DMA'd last.

Quantize (= mem_relu, exact round-half-down):
  ACT:  a = Relu(psum + (16*b - c))        c = 1/64 (0 for conv1)
  MID:  a = min(a, 64.5) + 1.5*2^23        (fp32 add rounds to integer)
  DVE:  out_bf16 = a - 1.5*2^23
Max-pool runs BEFORE quantize directly on the relu'd activations (max
commutes with the monotone bias/quantize chain; results identical to the
reference's quantize-then-pool order), so only pooled elements pay the
quantize cost.
"""
import sys
sys.path.insert(0, '/opt/trn_rl_repo')

import contextlib
import numpy as np
import ml_dtypes

import concourse.bass as bass
import concourse.tile as tile
from concourse import bacc, mybir, bass_utils

F32 = mybir.dt.float32
BF16 = mybir.dt.bfloat16
AF = mybir.ActivationFunctionType
OP = mybir.AluOpType
BF = ml_dtypes.bfloat16

M_MAGIC = 12582912.0      # 1.5*2^23
C_TIE = 1.0 / 64.0
B = 64                    # images per core
NCORES = 8
MG1, MG3, MG5 = 35, 19, 11
XCOLS = 2 * MG1 + 16 * 1156


def lut_quantize_np(v, lut):
    L = lut.shape[0]
    idx = np.clip(np.searchsorted(lut, v), 1, L - 1)
    left = lut[idx - 1]
    right = lut[idx]
    return np.where(v - left <= right - v, left, right).astype(np.float32)


def bf16_split3(x):
    h1 = x.astype(BF).astype(np.float32)
    r1 = x - h1
    h2 = r1.astype(BF).astype(np.float32)
    r2 = r1 - h2
    return h1, h2, r2


_CACHE = {}


def build_program(debug=False):
    key = ('prog', debug)
    if key in _CACHE:
        return _CACHE[key]

    nc = bacc.Bacc('TRN2', target_bir_lowering=False, debug=False)

    xfd = nc.dram_tensor('xfull', [128, XCOLS], BF16, kind='ExternalInput')
    w1d = nc.dram_tensor('w1', [128, 3 * 128], BF16, kind='ExternalInput')
    w2d = nc.dram_tensor('w2', [128, 9 * 128], BF16, kind='ExternalInput')
    w3d = nc.dram_tensor('w3', [128, 6 * 128], BF16, kind='ExternalInput')
    w4d = nc.dram_tensor('w4', [128, 9 * 128], BF16, kind='ExternalInput')
    w5d = nc.dram_tensor('w5', [128, 6 * 128], BF16, kind='ExternalInput')
    w6d = nc.dram_tensor('w6', [128, 9 * 128], BF16, kind='ExternalInput')
    fw1d = nc.dram_tensor('fw1', [128, 64 * 128], BF16, kind='ExternalInput')
    fw2d = nc.dram_tensor('fw2', [128, 4 * 128], BF16, kind='ExternalInput')
    fw3d = nc.dram_tensor('fw3', [128, 10], BF16, kind='ExternalInput')
    biasesd = nc.dram_tensor('biases', [128, 13], F32, kind='ExternalInput')
    fb1rd = nc.dram_tensor('fb1row', [1, 512], BF16, kind='ExternalInput')
    fb3rd = nc.dram_tensor('fb3r', [B, 10], F32, kind='ExternalInput')
    probsd = nc.dram_tensor('probs', [B, 10], F32, kind='ExternalOutput')

    dbg = {}
    if debug:
        for name, cols in [('dA1', XCOLS), ('dA2', 2 * MG3 + 16 * 324),
                           ('dA3', 2 * MG3 + 32 * 324),
                           ('dA4', 2 * MG5 + 32 * 100),
                           ('dA5', 2 * MG5 + 64 * 100),
                           ('dA6', 64 * 16), ('dA7', 4 * 64), ('dA8', 64)]:
            dbg[name] = nc.dram_tensor(name, [128, cols], BF16,
                                       kind='ExternalOutput')

    with tile.TileContext(nc) as tc:
        with contextlib.ExitStack() as ctx:
            act = ctx.enter_context(tc.tile_pool(name='acts', bufs=1))
            wp = ctx.enter_context(tc.tile_pool(name='weights', bufs=1))
            pp = ctx.enter_context(tc.tile_pool(name='psum', bufs=2, space='PSUM'))
            ev = ctx.enter_context(tc.tile_pool(name='evac', bufs=3))
            pl = ctx.enter_context(tc.tile_pool(name='pool', bufs=2))

            wt1 = wp.tile([128, 3 * 128], BF16, tag='w1')
            wt2 = wp.tile([128, 9 * 128], BF16, tag='w2')
            wt3 = wp.tile([128, 6 * 128], BF16, tag='w3')
            wt4 = wp.tile([128, 9 * 128], BF16, tag='w4')
            wt5 = wp.tile([128, 6 * 128], BF16, tag='w5')
            wt6 = wp.tile([128, 9 * 128], BF16, tag='w6')
            fwt1 = wp.tile([128, 64 * 128], BF16, tag='fw1')
            fwt2 = wp.tile([128, 4 * 128], BF16, tag='fw2')
            fwt3 = wp.tile([128, 10], BF16, tag='fw3')
            bt = wp.tile([128, 13], F32, tag='biases')
            fb1t = wp.tile([1, 512], BF16, tag='fb1row')
            ones = wp.tile([1, B], BF16, tag='ones')
            fb3t = wp.tile([B, 10], F32, tag='fb3r')
            Xt = act.tile([128, XCOLS], BF16, tag='X')
            # DMA order matters: the DMA engines are serialized, so issue
            # X chunk 0 + conv1 weights first (conv1 can then start ~5us in),
            # remaining X chunks next, and late-needed weights (fw1 is 2MB)
            # last. conv1 round r only needs image cols 4r:4r+4.
            def xchunk(r):
                lo = MG1 + 4 * r * 1156
                nc.sync.dma_start(Xt[:, lo: lo + 4 * 1156],
                                  xfd.ap()[:, lo: lo + 4 * 1156])
            nc.sync.dma_start(wt1[:], w1d.ap())
            xchunk(0)
            nc.sync.dma_start(bt[:], biasesd.ap())
            for r in range(1, 4):
                xchunk(r)
            for dram, sb in [(w2d, wt2), (w3d, wt3), (w4d, wt4),
                             (w5d, wt5), (w6d, wt6), (fw2d, fwt2),
                             (fw3d, fwt3), (fb3rd, fb3t), (fb1rd, fb1t),
                             (fw1d, fwt1)]:
                nc.sync.dma_start(sb[:], dram.ap())
            nc.gpsimd.memset(ones[:], 1.0)
            btiny = ev.tile([128, 1], F32, tag='btiny')
            nc.scalar.copy(btiny[:], bt[:, 0:1])
            ftiny = ev.tile([B, 1], F32, tag='ftiny')
            nc.scalar.copy(ftiny[:], fb3t[:, 0:1])

            # ---------------- helpers ----------------
            def memset_frame(t, n_img, W, marg):
                S = W * W
                nc.gpsimd.memset(t[:, 0:marg], 0.0)
                nc.gpsimd.memset(t[:, marg + n_img * S: marg + n_img * S + marg], 0.0)
                body = t[:, marg: marg + n_img * S].rearrange(
                    'p (u c) -> p u c', u=n_img)
                nc.gpsimd.memset(body[:, :, 0:W], 0.0)
                nc.gpsimd.memset(body[:, :, (W - 1) * W: W * W], 0.0)

            def memset_xp(t, n_img, W, marg, u0=None, u1=None):
                if u0 is None:
                    u0, u1 = 0, n_img
                S = W * W
                v = t[:, marg + u0 * S + W - 1: marg + u1 * S + W - 1].rearrange(
                    'p (u c) -> p u c', u=u1 - u0)[:, :, 0:(W - 1) * W]
                v = v.rearrange('p u (r c) -> p u r c', r=W - 1)[:, :, :, 0:2]
                nc.gpsimd.memset(v, 0.0)

            def evacuate(ps_view, n, bias_col, dst_ap, dst_dims=None,
                         act_final=False, mid=None):
                """Quantize n elems/partition from ps_view into dst_ap."""
                a = ev.tile([128, 2048], F32, tag='ev_a')
                nc.scalar.activation(a[:, 0:n], ps_view, AF.Relu,
                                     bias=bt[:, bias_col:bias_col + 1], scale=1.0)
                (mid or nc.vector).tensor_scalar(a[:, 0:n], a[:, 0:n], 64.5,
                                                 M_MAGIC, OP.min, OP.add)
                src = a[:, 0:n]
                if dst_dims is not None:
                    spec = ' '.join(f'd{i}' for i in range(len(dst_dims)))
                    src = src.rearrange(f'p ({spec}) -> p {spec}',
                                        **{f'd{i}': d for i, d in
                                           enumerate(dst_dims)})
                if act_final:
                    nc.scalar.activation(dst_ap, src, AF.Copy,
                                         bias=-M_MAGIC, scale=1.0)
                else:
                    nc.vector.tensor_scalar(dst_ap, src, M_MAGIC, None,
                                            OP.subtract)

            # ---------------- conv1: X -> A1 ----------------
            # Block-diagonal [128x128] matmuls: K = 4 strips x 27 rows
            # (split,dy,ch), M = 4 strips x 32 co. One matmul per
            # (bank, dx); bank b <-> image column u = 4r + b; two
            # half-height rounds (hh) of exactly 512 = 16y x 32x cols.
            # A1 map: image m = 4u + t at partition 32t+co, col u.
            A1 = act.tile([128, XCOLS], BF16, tag='A1')
            memset_frame(A1, 16, 34, MG1)
            memset_xp(A1, 16, 34, MG1)
            Xg = Xt[:, MG1: MG1 + 16 * 1156].rearrange(
                'p (u y x) -> p u y x', u=16, y=34)
            A1g = A1[:, MG1: MG1 + 16 * 1156].rearrange(
                'p (u y x) -> p u y x', u=16, y=34)
            for r in range(4):
                for hh in range(2):
                    ps = pp.tile([128, 2048], F32, tag='ps')
                    for b in range(4):
                        u = 4 * r + b
                        for dx in range(3):
                            nc.tensor.matmul(
                                ps[:, 512 * b: 512 * b + 512],
                                wt1[:, 128 * dx: 128 * dx + 128],
                                Xg[:, u, 16 * hh + 1: 16 * hh + 17, dx: dx + 32],
                                start=(dx == 0), stop=(dx == 2))
                    dst = A1g[:, 4 * r: 4 * r + 4,
                              16 * hh + 1: 16 * hh + 17, 1:33]
                    evacuate(ps[:], 2048, 0, dst, dst_dims=(4, 16, 32),
                             mid=nc.gpsimd if hh else nc.vector)
            if debug:
                nc.sync.dma_start(dbg['dA1'].ap(), A1[:])

            # ---------------- conv2: A1 -> (pool) -> A2 ----------------
            # Full K=128 block-diag, one matmul per (bank=image, tap),
            # 512 = 16y x 32x cols per bank, two half-height rounds.
            A2 = act.tile([128, 2 * MG3 + 16 * 324], BF16, tag='A2')
            memset_frame(A2, 16, 18, MG3)
            memset_xp(A2, 16, 18, MG3)
            A2g = A2[:, MG3: MG3 + 16 * 324].rearrange(
                'p (u y x) -> p u y x', u=16, y=18)
            for q in range(4):
                for hh in range(2):
                    ps = pp.tile([128, 2048], F32, tag='ps')
                    for b in range(4):
                        u = 4 * q + b
                        for tap in range(9):
                            dy, dx = tap // 3, tap % 3
                            nc.tensor.matmul(
                                ps[:, 512 * b: 512 * b + 512],
                                wt2[:, tap * 128: tap * 128 + 128],
                                A1g[:, u, 16 * hh + dy: 16 * hh + dy + 16,
                                    dx: dx + 32],
                                start=(tap == 0), stop=(tap == 8))
                    # relu+bias at full res (PSUM -> SBUF), then pool
                    af = ev.tile([128, 2048], F32, tag='ev_a')
                    nc.scalar.activation(af[:], ps[:], AF.Relu,
                                         bias=bt[:, 1:2], scale=1.0)
                    g = af[:].rearrange('p (t y x) -> p t y x', t=4, y=16)
                    p1 = pl.tile([128, 4 * 16 * 16], F32, tag='p1')
                    p1v = p1[:].rearrange('p (t y k) -> p t y k', t=4, y=16)
                    nc.vector.tensor_tensor(p1v, g[:, :, :, 0:32:2],
                                            g[:, :, :, 1:32:2], OP.max)
                    p2 = pl.tile([128, 4 * 8 * 16], F32, tag='p2')
                    p2v = p2[:].rearrange('p (t y k) -> p t y k', t=4, y=8)
                    nc.vector.tensor_tensor(p2v, p1v[:, :, 0:16:2, :],
                                            p1v[:, :, 1:16:2, :], OP.max)
                    dst = A2g[:, 4 * q: 4 * q + 4,
                              8 * hh + 1: 8 * hh + 9, 1:17]
                    nc.gpsimd.tensor_scalar(p2[:], p2[:], 64.5,
                                            M_MAGIC, OP.min, OP.add)
                    nc.vector.tensor_scalar(dst, p2v[:], M_MAGIC, None,
                                            OP.subtract)
            if debug:
                nc.sync.dma_start(dbg['dA2'].ap(), A2[:])

            # ---------------- conv3: A2 -> A3 ----------------
            # Pass h reads A2 strips {2h, 2h+1} (partitions 64h:64h+64,
            # images m = 4v + 2h + a), block-diag lhsT maps strip-s rows to
            # out cols 64*(s%2). Out ps[64a+co] = image 4v + 2h + a.
            # A3 map: image m at partition 64*(m%2)+co, col m//2.
            # Tap-pairing: replica partitions hold A2 shifted +18 (one frame
            # row), so K=128 = [base a0 | rep a0 | base a1 | rep a1] x 32ch
            # covers taps (dy, dy+1) in one pass: 6 passes instead of 9.
            # Replicas live in Xt's dead SBUF space, built by SBUF DMA.
            A2P = [Xt[:, 0:5184], Xt[:, 5184:10368]]
            for h in range(2):
                for half in range(2):
                    c0 = half * 2592
                    P = A2P[h]
                    for a in range(2):
                        srow = 64 * h + 32 * a
                        nc.sync.dma_start(
                            P[64 * a: 64 * a + 32, c0: c0 + 2592],
                            A2[srow: srow + 32, MG3 + c0: MG3 + c0 + 2592])
                        nc.sync.dma_start(
                            P[64 * a + 32: 64 * a + 64, c0: c0 + 2592],
                            A2[srow: srow + 32,
                               MG3 + c0 + 18: MG3 + c0 + 2592 + 18])
            A3 = act.tile([128, 2 * MG3 + 32 * 324], BF16, tag='A3')
            memset_frame(A3, 32, 18, MG3)
            memset_xp(A3, 32, 18, MG3)
            A3G = A3[:, MG3: MG3 + 32 * 324].rearrange(
                'p (u y x) -> p u y x', u=32, y=18)
            for vp in range(8):
                ps = pp.tile([128, 2048], F32, tag='ps')
                for dv in range(2):
                    v = 2 * vp + dv
                    for h in range(2):
                        Pg = A2P[h].rearrange('p (v y x) -> p v y x', v=16, y=18)
                        for j in range(6):
                            if j < 3:
                                rhs = Pg[:, v, 0:16, j: j + 16]
                            else:
                                rhs = Pg[:, v, 2:18, j - 3: j - 3 + 16]
                            nc.tensor.matmul(
                                ps[:, 512 * (2 * dv + h): 512 * (2 * dv + h) + 256],
                                wt3[:, j * 128: j * 128 + 128],
                                rhs, start=(j == 0), stop=(j == 5))
                psv = ps[:].rearrange('p (t c) -> p t c', t=4)[:, :, 0:256]
                dst = A3G[:, 4 * vp: 4 * vp + 4, 1:17, 1:17]
                evacuate(psv, 4 * 256, 2, dst, dst_dims=(4, 16, 16),
                         mid=nc.gpsimd if vp % 2 else nc.vector)
            if debug:
                nc.sync.dma_start(dbg['dA3'].ap(), A3[:])

            # ---------------- conv4: A3 -> (pool) -> A4 ----------------
            # Full K=128 block-diag (2 x [64,64]): one matmul per (col, tap);
            # bank i <-> A3 col vv+i (images m = 2*(vv+i) + a at 64a+co).
            # A4 map: image m at partition 64*(m%2)+co, col m//2 (= baseline).
            A4 = act.tile([128, 2 * MG5 + 32 * 100 + 24], BF16, tag='A4')
            memset_frame(A4, 32, 10, MG5)
            memset_xp(A4, 32, 10, MG5)
            A4g = A4[:, MG5: MG5 + 32 * 100].rearrange(
                'p (u y x) -> p u y x', u=32, y=10)
            for vv in range(0, 32, 4):
                ps = pp.tile([128, 2048], F32, tag='ps')
                for i in range(4):
                    for tap in range(9):
                        dy, dx = tap // 3, tap % 3
                        nc.tensor.matmul(
                            ps[:, 512 * i: 512 * i + 256],
                            wt4[:, tap * 128: tap * 128 + 128],
                            A3G[:, vv + i, dy: dy + 16, dx: dx + 16],
                            start=(tap == 0), stop=(tap == 8))
                psv = ps[:].rearrange('p (b c) -> p b c', b=4)[:, :, 0:256]
                af = ev.tile([128, 2048], F32, tag='ev_a')
                nc.scalar.activation(af[:, 0:1024], psv, AF.Relu,
                                     bias=bt[:, 3:4], scale=1.0)
                g = af[:, 0:1024].rearrange('p (i y x) -> p i y x',
                                            i=4, y=16)
                p1 = pl.tile([128, 4 * 16 * 8], F32, tag='p41')
                p1v = p1[:].rearrange('p (i y k) -> p i y k', i=4, y=16)
                nc.vector.tensor_tensor(p1v, g[:, :, :, 0:16:2],
                                        g[:, :, :, 1:16:2], OP.max)
                p2 = pl.tile([128, 4 * 8 * 8], F32, tag='p42')
                p2v = p2[:].rearrange('p (i y k) -> p i y k', i=4, y=8)
                nc.vector.tensor_tensor(p2v, p1v[:, :, 0:16:2, :],
                                        p1v[:, :, 1:16:2, :], OP.max)
                nc.gpsimd.tensor_scalar(p2[:], p2[:], 64.5, M_MAGIC,
                                        OP.min, OP.add)
                dst = A4g[:, vv: vv + 4, 1:9, 1:9]
                nc.vector.tensor_scalar(
                    dst, p2v[:], M_MAGIC, None, OP.subtract)
            if debug:
                nc.sync.dma_start(dbg['dA4'].ap(), A4[:, 0:2 * MG5 + 32 * 100])

            # ---------------- conv5: A4 -> A5 ----------------
            # Same tap-pairing as conv3: K=128 = [base ci | rep(+10) ci],
            # 6 passes; replicas in A1's dead SBUF space.
            A4P = [A1[:, 0:3200], A1[:, 3200:6400]]
            for t in range(2):
                for half in range(4):
                    c0 = half * 800
                    P = A4P[t]
                    nc.sync.dma_start(
                        P[0:64, c0: c0 + 800],
                        A4[64 * t: 64 * t + 64, MG5 + c0: MG5 + c0 + 800])
                    nc.sync.dma_start(
                        P[64:128, c0: c0 + 800],
                        A4[64 * t: 64 * t + 64,
                           MG5 + c0 + 10: MG5 + c0 + 800 + 10])
            A5 = act.tile([128, 2 * MG5 + 64 * 100 + 24], BF16, tag='A5')
            memset_frame(A5, 64, 10, MG5)
            memset_xp(A5, 64, 10, MG5)
            A5G = A5[:, MG5: MG5 + 64 * 100].rearrange(
                'p (t u y x) -> p t u y x', t=2, u=32, y=10)
            for qp in range(4):
                ps = pp.tile([128, 2048], F32, tag='ps')
                for qd in range(2):
                    q = 2 * qp + qd
                    for t in range(2):
                        Pg = A4P[t].rearrange('p (u y x) -> p u y x', u=32, y=10)
                        for j in range(6):
                            if j < 3:
                                rhs = Pg[:, 4 * q: 4 * q + 4, 0:8, j: j + 8]
                            else:
                                rhs = Pg[:, 4 * q: 4 * q + 4, 2:10,
                                         j - 3: j - 3 + 8]
                            nc.tensor.matmul(
                                ps[:, 512 * (2 * qd + t): 512 * (2 * qd + t) + 256],
                                wt5[:, j * 128: j * 128 + 128],
                                rhs, start=(j == 0), stop=(j == 5))
                psv = ps[:].rearrange('p (t c) -> p t c', t=4)[:, :, 0:256]
                a5a = ev.tile([128, 2048], F32, tag='ev_a')
                nc.scalar.activation(a5a[:, 0:1024], psv, AF.Relu,
                                     bias=bt[:, 4:5], scale=1.0)
                nc.vector.tensor_scalar(a5a[:, 0:1024], a5a[:, 0:1024], 64.5,
                                        M_MAGIC, OP.min, OP.add)
                for t in range(2):
                    srcv = a5a[:, 0:1024].rearrange(
                        'p (qd t u y x) -> p qd t u y x', qd=2, t=2, u=4,
                        y=8)[:, :, t]
                    nc.vector.tensor_scalar(
                        A5G[:, t, 8 * qp: 8 * qp + 8, 1:9, 1:9].rearrange(
                            'p (qd u) y x -> p qd u y x', qd=2),
                        srcv, M_MAGIC, None, OP.subtract)
            if debug:
                nc.sync.dma_start(dbg['dA5'].ap(), A5[:, 0:2 * MG5 + 64 * 100])

            # ---------------- conv6: A5 -> (pool) -> A6 ----------------
            A6 = act.tile([128, 64 * 16], BF16, tag='A6')
            A5F = A5[:, MG5: MG5 + 64 * 100].rearrange(
                'p (w y x) -> p w y x', w=64, y=10)
            for qq in range(0, 16, 2):
                ps = pp.tile([128, 2048], F32, tag='ps')
                for i in range(2):
                    q = qq + i
                    for tap in range(9):
                        dy, dx = tap // 3, tap % 3
                        nc.tensor.matmul(
                            ps[:, 512 * i: 512 * i + 256],
                            wt6[:, tap * 128: tap * 128 + 128],
                            A5F[:, 4 * q: 4 * q + 4, dy: dy + 8, dx: dx + 8],
                            start=(tap == 0), stop=(tap == 8))
                psv = ps[:, 0:1024].rearrange('p (t c) -> p t c', t=2)[:, :, 0:256]
                af = ev.tile([128, 2048], F32, tag='ev_a')
                nc.scalar.activation(af[:, 0:512], psv, AF.Relu,
                                     bias=bt[:, 5:6], scale=1.0)
                g = af[:, 0:512].rearrange('p (t i y x) -> p t i y x',
                                           t=2, i=4, y=8)
                p1 = pl.tile([128, 2 * 4 * 8 * 4], F32, tag='p61')
                p1v = p1[:].rearrange('p (t i y k) -> p t i y k', t=2, i=4, y=8)
                nc.vector.tensor_tensor(p1v, g[:, :, :, :, 0:8:2],
                                        g[:, :, :, :, 1:8:2], OP.max)
                p2 = pl.tile([128, 2 * 4 * 4 * 4], F32, tag='p62')
                p2v = p2[:].rearrange('p (t i y k) -> p t i y k', t=2, i=4, y=4)
                nc.vector.tensor_tensor(p2v, p1v[:, :, :, 0:8:2, :],
                                        p1v[:, :, :, 1:8:2, :], OP.max)
                dst = A6[:, 4 * qq * 16: 4 * qq * 16 + 128]
                nc.gpsimd.tensor_scalar(p2[:], p2[:], 64.5, M_MAGIC,
                                        OP.min, OP.add)
                nc.vector.tensor_scalar(dst, p2[:], M_MAGIC, None, OP.subtract)
            if debug:
                nc.sync.dma_start(dbg['dA6'].ap(), A6[:])

            # ---------------- fc1 ----------------
            A7 = act.tile([128, 4 * 64], BF16, tag='A7')
            a6s = A6[:].rearrange('p (i s) -> p i s', s=16)
            ps = pp.tile([128, 2048], F32, tag='ps')
            for mchunk in range(4):
                for s in range(16):
                    nc.tensor.matmul(
                        ps[:, 512 * mchunk: 512 * mchunk + 64],
                        fwt1[:, (s * 4 + mchunk) * 128: (s * 4 + mchunk) * 128 + 128],
                        a6s[:, :, s],
                        start=(s == 0), stop=False)
                nc.tensor.matmul(
                    ps[:, 512 * mchunk: 512 * mchunk + 64],
                    fb1t[0:1, 128 * mchunk: 128 * mchunk + 128],
                    ones[0:1, 0:B],
                    start=False, stop=True)
            psv = ps[:].rearrange('p (m c) -> p m c', m=4)[:, :, 0:64]
            evacuate(psv, 256, 12, A7[:])
            if debug:
                nc.sync.dma_start(dbg['dA7'].ap(), A7[:])

            # ---------------- fc2 ----------------
            A8 = act.tile([128, 64], BF16, tag='A8')
            ps = pp.tile([128, 2048], F32, tag='ps')
            for kchunk in range(4):
                nc.tensor.matmul(
                    ps[:, 0:64], fwt2[:, kchunk * 128: kchunk * 128 + 128],
                    A7[:, kchunk * 64: kchunk * 64 + 64],
                    start=(kchunk == 0), stop=(kchunk == 3))
            evacuate(ps[:, 0:64], 64, 10, A8[:])
            if debug:
                nc.sync.dma_start(dbg['dA8'].ap(), A8[:])

            # ---------------- fc3 + softmax ----------------
            ps3 = pp.tile([128, 2048], F32, tag='ps')
            nc.tensor.matmul(ps3[0:64, 0:10], A8[:, 0:64], fwt3[:, 0:10],
                             start=True, stop=True)
            logits = ev.tile([B, 10], F32, tag='sm_l')
            nc.vector.tensor_tensor(logits[:], ps3[0:64, 0:10], fb3t[:], OP.add)
            mx = ev.tile([B, 1], F32, tag='sm_m')
            nc.vector.tensor_reduce(mx[:], logits[:], mybir.AxisListType.X, OP.max)
            nmx = ev.tile([B, 1], F32, tag='sm_n')
            nc.vector.tensor_scalar(nmx[:], mx[:], -1.0, None, OP.mult)
            exps = ev.tile([B, 10], F32, tag='sm_e')
            nc.scalar.activation(exps[:], logits[:], AF.Exp,
                                 bias=nmx[:, 0:1], scale=1.0)
            sm = ev.tile([B, 1], F32, tag='sm_s')
            nc.vector.tensor_reduce(sm[:], exps[:], mybir.AxisListType.X, OP.add)
            rec = ev.tile([B, 1], F32, tag='sm_r')
            nc.vector.reciprocal(rec[:], sm[:])
            prsb = ev.tile([B, 10], F32, tag='sm_p')
            nc.vector.tensor_scalar(prsb[:], exps[:], rec[:, 0:1], None, OP.mult)
            nc.sync.dma_start(probsd.ap(), prsb[:])

    nc.compile()
    _CACHE[key] = nc
    return nc


# ---------------------------------------------------------------------------
# host-side preparation
# ---------------------------------------------------------------------------

def prep_weights(lut, ws, bs, fws, fbs):
    lut = np.asarray(lut, np.float32)
    q = lambda a: lut_quantize_np(np.asarray(a, np.float32), lut)
    w1q, w2q, w3q, w4q, w5q, w6q = [q(w) for w in ws]
    b1q, b2q, b3q, b4q, b5q, b6q = [q(b) for b in bs]
    fw1q, fw2q, fw3q = [q(w) for w in fws]
    fb1q, fb2q, fb3q = [q(b) for b in fbs]

    out = {}
    # conv1 block-diag: rows 32t + (9s+3k+ch), cols 128dx + 32t + co
    w1a = np.zeros((128, 3 * 128), np.float32)
    for t in range(4):
        for s in range(3):
            for k in range(3):
                for ch in range(3):
                    p = 32 * t + 9 * s + 3 * k + ch
                    for dx in range(3):
                        w1a[p, 128 * dx + 32 * t: 128 * dx + 32 * t + 32] = \
                            16.0 * w1q[:, ch, k, dx]
    out['w1'] = w1a.astype(BF)

    def conv_w_bd(wq, Ci, Co, nstrip, ostride):
        """Block-diag: rows Ci*t + ci -> cols 128*tap + ostride*t + co."""
        a = np.zeros((128, 9 * 128), np.float32)
        for t in range(nstrip):
            for tap in range(9):
                dy, dx = tap // 3, tap % 3
                a[Ci * t: Ci * t + Ci,
                  tap * 128 + ostride * t: tap * 128 + ostride * t + Co] = \
                    wq[:, :, dy, dx].T
        return a.astype(BF)

    def conv_w(wq, Ci, Co, nstrip, stripsz):
        a = np.zeros((128, 9 * Co), np.float32)
        for t in range(nstrip):
            for tap in range(9):
                dy, dx = tap // 3, tap % 3
                a[stripsz * t: stripsz * t + Ci, tap * Co: tap * Co + Co] = \
                    wq[:, :, dy, dx].T
        return a.astype(BF)

    out['w2'] = conv_w_bd(w2q, 32, 32, 4, 32)
    # conv3 tap-paired: K rows [base a0 | rep a0 | base a1 | rep a1] x 32ch,
    # out col-block 64a. Pass j<3: (dy=0 base, dy=1 rep, dx=j); j>=3:
    # (dy=2 base only, dx=j-3).
    w3a = np.zeros((128, 6 * 128), np.float32)
    for a in range(2):
        for j in range(3):
            c = 64 * a
            w3a[64 * a: 64 * a + 32, j * 128 + c: j * 128 + c + 64] = \
                w3q[:, :, 0, j].T
            w3a[64 * a + 32: 64 * a + 64, j * 128 + c: j * 128 + c + 64] = \
                w3q[:, :, 1, j].T
            w3a[64 * a: 64 * a + 32, (3 + j) * 128 + c: (3 + j) * 128 + c + 64] = \
                w3q[:, :, 2, j].T
    out['w3'] = w3a.astype(BF)
    out['w4'] = conv_w_bd(w4q, 64, 64, 2, 64)
    # conv5 tap-paired: K rows [base ci | rep ci], dense M=128
    w5a = np.zeros((128, 6 * 128), np.float32)
    for j in range(3):
        w5a[0:64, j * 128: j * 128 + 128] = w5q[:, :, 0, j].T
        w5a[64:128, j * 128: j * 128 + 128] = w5q[:, :, 1, j].T
        w5a[0:64, (3 + j) * 128: (3 + j) * 128 + 128] = w5q[:, :, 2, j].T
    out['w5'] = w5a.astype(BF)
    out['w6'] = conv_w(w6q, 128, 128, 1, 128)

    f1 = fw1q.reshape(4, 128, 128, 16)
    out['fw1'] = np.ascontiguousarray(
        np.transpose(f1, (2, 3, 0, 1)).reshape(128, 64 * 128)).astype(BF)
    f2 = fw2q.reshape(128, 4, 128)
    out['fw2'] = np.ascontiguousarray(
        np.transpose(f2, (2, 1, 0)).reshape(128, 512)).astype(BF)
    out['fw3'] = np.ascontiguousarray((fw3q.T / 16.0)).astype(BF)

    biases = np.zeros((128, 13), np.float32)
    p = np.arange(128)
    biases[:, 0] = 16.0 * b1q[p % 32]
    biases[:, 1] = 16.0 * b2q[p % 32] - C_TIE
    biases[:, 2] = 16.0 * b3q[p % 64] - C_TIE
    biases[:, 3] = 16.0 * b4q[p % 64] - C_TIE
    biases[:, 4] = 16.0 * b5q[p] - C_TIE
    biases[:, 5] = 16.0 * b6q[p] - C_TIE
    for m in range(4):
        biases[:, 6 + m] = 16.0 * fb1q[m * 128 + p] - C_TIE
    biases[:, 10] = 16.0 * fb2q[p] - C_TIE
    biases[:, 12] = -C_TIE
    out['biases'] = biases
    # fc1 bias added in-PSUM via a K=1 ones-row matmul (16*fb1q is int-exact
    # in bf16; the -C_TIE part stays in the ACT bias, col 12)
    out['fb1row'] = (16.0 * fb1q[None, :]).astype(BF)
    out['fb3r'] = np.tile(fb3q[None, :], (B, 1)).astype(np.float32)
    return out


def prep_x(x):
    """x [B,3,32,32] fp32 -> assembled Xt payload [128, XCOLS] bf16."""
    xp = np.zeros((x.shape[0], 3, 34, 34), np.float32)
    xp[:, :, 1:33, 1:33] = x
    h = bf16_split3(xp)
    X = np.zeros((128, XCOLS), np.float32)
    X2 = X[:, MG1: MG1 + 16 * 1156].reshape(128, 16, 1156)
    for t in range(4):
        imgs = np.arange(16) * 4 + t
        for s in range(3):
            sub = h[s][imgs]                # [16, 3, 34, 34]
            for k in range(3):
                srcv = sub[:, :, k:k + 32, :].reshape(16, 3, 1088)
                for ch in range(3):
                    X2[32 * t + 9 * s + 3 * k + ch, :, 34:1122] = srcv[:, ch]
    return np.ascontiguousarray(X.astype(BF))


POS_OF_M = np.array([32 * (m % 2) + m // 2 for m in range(B)])


def kernel(x, lut, w1, b1, w2, b2, w3, b3, w4, b4, w5, b5, w6, b6,
           fw1, fb1, fw2, fb2, fw3, fb3, _debug=False, _trace=False):
    x = np.asarray(x, np.float32)
    nimg = x.shape[0]
    assert nimg == NCORES * B

    nc = build_program(debug=_debug)
    shared = prep_weights(lut, [w1, w2, w3, w4, w5, w6],
                          [b1, b2, b3, b4, b5, b6],
                          [fw1, fw2, fw3], [fb1, fb2, fb3])

    in_maps = []
    for c in range(NCORES):
        m = dict(shared)
        m['xfull'] = prep_x(x[c * B:(c + 1) * B])
        in_maps.append(m)

    res = bass_utils.run_bass_kernel_spmd(
        nc, in_maps, core_ids=list(range(NCORES)), trace=_trace)

    out = np.zeros((nimg, 10), np.float32)
    for c in range(NCORES):
        dev = res.results[c]['probs']
        out[c * B:(c + 1) * B] = dev[POS_OF_M]
    if _debug or _trace:
        kernel._last = res
    return out

